# revision 1
# baseline (speedup 1.0000x reference)
"""Multi-head attention (B=2, F=T=2048, 16 heads x 64) on 8 TRN2 NeuronCores.

Sharding: core c = (batch b = c//4) x (head-group g = c%4, 4 heads each).
Each core computes, for its batch and its 4 heads:
    q = X @ Wq + bq ; k = Y @ Wk + bk ; v = Y @ Wv + bv
    probs = softmax(q k^T / 8 + mask_adder) ; ctx = probs @ v
entirely on-device; host only shards/transposes inputs and gathers outputs.

Device layouts (bf16 compute, f32 PSUM/softmax):
  xt  [1024, F]  = from[b]^T          yt [1024, T] = to[b]^T
  wq/wk/wv [1025, 256] = [W[:, g*256:(g+1)*256] ; bias row]
  maskT [T, F] = mask[b]^T (0/1 as bf16)
  out [256, F] f32 = ctx^T for the head group
Per head h: scoresT[t,f] = kh^T qh (K=64 matmul), e = exp(0.125*s) (ScalarE,
mask applied multiplicatively on VectorE since exp(s-10000*(1-m)) == exp(s)*m),
ctx^T and the softmax denominator come from one PSUM-accumulated matmul with
lhsT = [vh | ones] (M=65); normalize via reciprocal + K=1 replicate matmul.
"""
import sys
import numpy as np

for _p in ("/opt/trn_rl_repo",):
    if _p not in sys.path:
        sys.path.insert(0, _p)

import ml_dtypes

bf16 = ml_dtypes.bfloat16

N_HEADS_TOTAL = 16
HEAD_DIM = 64
HIDDEN = N_HEADS_TOTAL * HEAD_DIM
N_CORES = 8
HEADS_PER_CORE = N_HEADS_TOTAL // 4  # 4 heads per core (4 head-groups)


def build_nc(F=2048, T=2048, D=1024, NH=4, NBLK=512, EXP_CHUNK=1024, reps=1):
    """Build the per-core Bass graph. All dims must divide evenly.

    reps>1 wraps the whole body in a device-side For_i loop (used only for
    timing measurements: one host call then runs the body `reps` times)."""
    from contextlib import nullcontext
    from concourse import bass, bacc, tile, mybir

    f32 = mybir.dt.float32
    b16 = mybir.dt.bfloat16
    KT = D // 128           # contraction k-tiles
    TT = T // 128           # t tiles
    FB = F // NBLK          # f blocks for matmul N
    MT = (NH * HEAD_DIM) // 128  # output-channel tiles for q/k (2 when NH=4)
    DG = NH * HEAD_DIM      # 256
    EXP_CHUNK = min(EXP_CHUNK, F)
    EC = F // EXP_CHUNK

    nc = bacc.Bacc(None, target_bir_lowering=False, debug=False)

    xt_d = nc.declare_dram_parameter("xt", [D, F], b16, isOutput=False)
    yt_d = nc.declare_dram_parameter("yt", [D, T], b16, isOutput=False)
    mk_d = nc.declare_dram_parameter("maskT", [T, F], b16, isOutput=False)
    wq_d = nc.declare_dram_parameter("wq", [D + 1, DG], b16, isOutput=False)
    wk_d = nc.declare_dram_parameter("wk", [D + 1, DG], b16, isOutput=False)
    wv_d = nc.declare_dram_parameter("wv", [D + 1, DG], b16, isOutput=False)
    out_d = nc.declare_dram_parameter("out", [DG, F], f32, isOutput=True)

    with tile.TileContext(nc) as tc:
        with (
            tc.tile_pool(name="res", bufs=1) as res,
            tc.tile_pool(name="epool", bufs=6) as epool,
            tc.tile_pool(name="empool", bufs=8) as empool,
            tc.tile_pool(name="npool", bufs=1) as npool,
            tc.tile_pool(name="psc", bufs=1, space="PSUM") as pspc,
            tc.tile_pool(name="pss", bufs=2, space="PSUM") as psps,
            tc.For_i(0, reps, 1) if reps > 1 else nullcontext(),
        ):
            # ---- resident SBUF tensors ----
            xt_sb = res.tile([128, KT, F], b16, tag="xtmask")     # released after q-proj
            yt_sb = res.tile([128, KT, T], b16, tag="yt")
            mask_lo = res.tile([128, TT // 2, F], b16, tag="mlo")
            wq_sb = res.tile([128, KT, DG], b16, tag="wq")
            wk_sb = res.tile([128, KT, DG], b16, tag="wk")
            wv_sb = res.tile([128, KT, DG], b16, tag="wv")
            wqb = res.tile([1, DG], b16, tag="wqb")
            wkb = res.tile([1, DG], b16, tag="wkb")
            wvb = res.tile([1, DG], b16, tag="wvb")
            ones_f = res.tile([1, max(F, T)], b16, tag="ones")    # ones row
            ones64 = res.tile([1, 64], f32, tag="ones64")
            qT_sb = res.tile([128, MT, F], b16, tag="qT")
            kT_sb = res.tile([128, MT, T], b16, tag="kT")
            v_sb = res.tile([128, TT, NH, HEAD_DIM + 1], b16, tag="v")

            nc.vector.memset(ones_f[:], 1.0)
            nc.vector.memset(ones64[:], 1.0)
            nc.vector.memset(v_sb[:, :, :, HEAD_DIM], 1.0)

            # ---- input DMAs (per-k/t granularity spreads across queues) ----
            for k in range(KT):
                nc.sync.dma_start(xt_sb[:, k, :], xt_d[k * 128:(k + 1) * 128, :])
            nc.sync.dma_start(wq_sb[:], wq_d[0:D, :].rearrange("(k p) n -> p k n", p=128))
            nc.sync.dma_start(wk_sb[:], wk_d[0:D, :].rearrange("(k p) n -> p k n", p=128))
            nc.sync.dma_start(wv_sb[:], wv_d[0:D, :].rearrange("(k p) n -> p k n", p=128))
            nc.sync.dma_start(wqb[:], wq_d[D:D + 1, :])
            nc.sync.dma_start(wkb[:], wk_d[D:D + 1, :])
            nc.sync.dma_start(wvb[:], wv_d[D:D + 1, :])
            for k in range(KT):
                nc.sync.dma_start(yt_sb[:, k, :], yt_d[k * 128:(k + 1) * 128, :])
            for t in range(TT // 2):
                nc.sync.dma_start(mask_lo[:, t, :], mk_d[t * 128:(t + 1) * 128, :])

            # ---- projections ----
            def blocks(n):
                return [slice(i, min(i + NBLK, n)) for i in range(0, n, NBLK)]

            HF = F // 2  # f-half width (PSUM s-slot size)

            def proj_qk(dst_sb, w_sb, w_b, act_sb, act1, n_cols):
                # dst[:, m, :] = (W^T @ act)[m-tile]   (weight-stationary)
                for m in range(MT):
                    for h0 in range(0, n_cols, HF):
                        hw = min(HF, n_cols - h0)
                        ps = psps.tile([128, HF], f32, tag="s")
                        for cs in blocks(hw):
                            gs = slice(h0 + cs.start, h0 + cs.stop)
                            for k in range(KT):
                                nc.tensor.matmul(
                                    ps[:, cs],
                                    w_sb[:, k, m * 128:(m + 1) * 128],
                                    act_sb[:, k, gs],
                                    start=(k == 0), stop=False,
                                )
                            nc.tensor.matmul(
                                ps[:, cs],
                                w_b[0:1, m * 128:(m + 1) * 128],
                                act1[0:1, gs],
                                start=False, stop=True,
                            )
                        nc.vector.tensor_copy(dst_sb[:, m, h0:h0 + hw], ps[:, :hw])

            proj_qk(qT_sb, wq_sb, wqb, xt_sb, ones_f, F)
            proj_qk(kT_sb, wk_sb, wkb, yt_sb, ones_f, T)

            # v natural layout [t, d] (activation-stationary)
            for t in range(TT):
                ps = psps.tile([128, DG], f32, tag="s")
                for k in range(KT):
                    nc.tensor.matmul(
                        ps[:], yt_sb[:, k, t * 128:(t + 1) * 128], wv_sb[:, k, :],
                        start=(k == 0), stop=False)
                nc.tensor.matmul(
                    ps[:], ones_f[0:1, t * 128:(t + 1) * 128], wvb[0:1, :],
                    start=False, stop=True)
                nc.vector.tensor_copy(v_sb[:, t, :, 0:HEAD_DIM], ps[:])

            # mask upper half reuses xt's slot once q-projection has consumed xt
            mask_hi = res.tile([128, TT - TT // 2, F], b16, tag="xtmask")
            for t in range(TT - TT // 2):
                nc.sync.dma_start(mask_hi[:, t, :],
                                  mk_d[(TT // 2 + t) * 128:(TT // 2 + t + 1) * 128, :])

            def mask_tile(t):
                return mask_lo[:, t, :] if t < TT // 2 else mask_hi[:, t - TT // 2, :]

            # ---- per-head attention ----
            EXPF = mybir.ActivationFunctionType.Exp

            for h in range(NH):
                hp = (h % 2) * 64          # partition offset within m-tile
                hm = h // 2                # which m-tile of qT/kT
                ctx_ps = pspc.tile([HEAD_DIM + 1, F], f32, tag="ctx")

                def ctx_mms(td, ems):
                    # ctx matmuls for data-step td (issued one t late so PE
                    # never stalls on the ACT->DVE chain of the same t)
                    for half in range(2):
                        h0 = half * HF
                        for cs in blocks(HF):
                            gs = slice(h0 + cs.start, h0 + cs.stop)
                            nc.tensor.matmul(
                                ctx_ps[:, gs], v_sb[:, td, h, :], ems[half][:, cs],
                                start=(td == 0), stop=(td == TT - 1))

                pend = None
                for t in range(TT):
                    kh = kT_sb[hp:hp + 64, hm, t * 128:(t + 1) * 128]
                    ems = []
                    for half in range(2):
                        h0 = half * HF
                        s_ps = psps.tile([128, HF], f32, tag="s")
                        for cs in blocks(HF):
                            gs = slice(h0 + cs.start, h0 + cs.stop)
                            nc.tensor.matmul(
                                s_ps[:, cs], kh, qT_sb[hp:hp + 64, hm, gs],
                                start=True, stop=True)
                        e_sb = epool.tile([128, HF], b16, tag="e")
                        em_sb = empool.tile([128, HF], b16, tag="em")
                        nc.scalar.activation(e_sb[:], s_ps[:], EXPF, scale=0.125)
                        nc.vector.tensor_mul(em_sb[:], e_sb[:],
                                             mask_tile(t)[:, h0:h0 + HF])
                        ems.append(em_sb)
                    if pend is not None:
                        ctx_mms(*pend)
                    pend = (t, ems)
                ctx_mms(*pend)
                # normalize: out[h] = ctx * (1/denom), broadcast over partitions
                o_sb = npool.tile([64, F], f32, tag="o")
                for half in range(2):
                    h0 = half * HF
                    r_sb = npool.tile([1, HF], f32, tag="r")
                    nc.vector.reciprocal(r_sb[:], ctx_ps[HEAD_DIM:HEAD_DIM + 1,
                                                         h0:h0 + HF])
                    rep_ps = psps.tile([64, HF], f32, tag="s")
                    for cs in blocks(HF):
                        nc.tensor.matmul(rep_ps[:, cs], ones64[0:1, :],
                                         r_sb[0:1, cs], start=True, stop=True)
                    rrep = npool.tile([64, HF], b16, tag="rrep")
                    nc.vector.tensor_copy(rrep[:], rep_ps[:])
                    nc.vector.tensor_mul(o_sb[:, h0:h0 + HF],
                                         ctx_ps[0:HEAD_DIM, h0:h0 + HF], rrep[:])
                nc.sync.dma_start(out_d[h * 64:(h + 1) * 64, :], o_sb[:])

    return nc


_CACHE = {}
TRACE = False  # set True (e.g. from test.py) to capture a neuron profile


def _get_nc():
    if "nc" not in _CACHE:
        nc = build_nc()
        nc.compile()
        _CACHE["nc"] = nc
    return _CACHE["nc"]


def prep_in_maps(from_tensor, to_tensor, attention_mask, Wq, bq, Wk, bk, Wv, bv):
    from_tensor = np.asarray(from_tensor, np.float32)
    to_tensor = np.asarray(to_tensor, np.float32)
    attention_mask = np.asarray(attention_mask)
    in_maps = []
    for c in range(N_CORES):
        b, g = c // 4, c % 4
        sl = slice(g * 256, (g + 1) * 256)
        wqa = np.concatenate([np.asarray(Wq, np.float32)[:, sl],
                              np.asarray(bq, np.float32)[None, sl]], 0)
        wka = np.concatenate([np.asarray(Wk, np.float32)[:, sl],
                              np.asarray(bk, np.float32)[None, sl]], 0)
        wva = np.concatenate([np.asarray(Wv, np.float32)[:, sl],
                              np.asarray(bv, np.float32)[None, sl]], 0)
        in_maps.append({
            "xt": np.ascontiguousarray(from_tensor[b].T).astype(bf16),
            "yt": np.ascontiguousarray(to_tensor[b].T).astype(bf16),
            "maskT": np.ascontiguousarray(
                attention_mask[b].T.astype(np.float32)).astype(bf16),
            "wq": wqa.astype(bf16),
            "wk": wka.astype(bf16),
            "wv": wva.astype(bf16),
        })
    return in_maps


def gather_out(per_core_outs, B, F):
    out = np.zeros((B, F, HIDDEN), np.float32)
    for c in range(N_CORES):
        b, g = c // 4, c % 4
        out[b, :, g * 256:(g + 1) * 256] = np.asarray(per_core_outs[c]).T
    return out


def kernel(from_tensor, to_tensor, attention_mask, Wq, bq, Wk, bk, Wv, bv):
    from concourse.bass_utils import run_bass_kernel_spmd

    B, F, _ = np.asarray(from_tensor).shape
    nc = _get_nc()
    in_maps = prep_in_maps(from_tensor, to_tensor, attention_mask,
                           Wq, bq, Wk, bk, Wv, bv)
    res = run_bass_kernel_spmd(nc, in_maps, core_ids=list(range(N_CORES)),
                               trace=TRACE)
    _CACHE["last_result"] = res
    return gather_out([res.results[c]["out"] for c in range(N_CORES)], B, F)



# revision 3
# speedup vs baseline: 3.0670x; 3.0670x over previous
"""Multi-head attention (B=2, F=T=2048, 16 heads x 64) on 8 TRN2 NeuronCores.

Sharding: core c = (batch b = c//4) x (head-group g = c%4, 4 heads each).

V1 design notes (vs baseline):
- ctx in natural layout: out[f, h] accumulated as matmul(lhsT=em[t, f-chunk],
  rhs=v[t, 65]) -> N=65 per matmul instead of N=512 streams of em; halves the
  PE rows for ctx and makes the softmax denominator a per-partition scalar.
- Softmax denominator from the 65th (ones) column of v; normalize via DVE
  reciprocal + tensor_scalar per-partition multiply (no replicate matmul).
- q/k biases fused into the PSUM->SBUF copies (tensor_scalar add), v bias
  via a K=1 ones-row matmul as before.
- Projections emitted interleaved into the attention loop as PE filler so the
  tensor engine never idles while ACT does exp (keeps the PE p-state at max).
- Output in natural layout [F, 256] bf16; single gather, no host transpose.
"""
import sys
import numpy as np

for _p in ("/opt/trn_rl_repo",):
    if _p not in sys.path:
        sys.path.insert(0, _p)

import ml_dtypes

bf16 = ml_dtypes.bfloat16

N_HEADS_TOTAL = 16
HEAD_DIM = 64
HIDDEN = N_HEADS_TOTAL * HEAD_DIM
N_CORES = 8
NH = 4  # heads per core


def build_nc(F=2048, T=2048, D=1024, reps=1):
    import os
    from contextlib import nullcontext
    from concourse import bass, bacc, tile, mybir

    KN = lambda name, d: int(os.environ.get('K_' + name, d))

    f32 = mybir.dt.float32
    b16 = mybir.dt.bfloat16
    KT = D // 128            # 8 contraction k-tiles
    TT = T // 128            # 16 t tiles
    MT = (NH * HEAD_DIM) // 128  # 2 output-channel m-tiles for q/k
    DG = NH * HEAD_DIM       # 256
    HF = F // 2              # 1024: scores psum half width
    NCH = F // 128           # 16 ctx f-chunks
    VW = HEAD_DIM + 1        # 65: v columns + ones

    nc = bacc.Bacc(None, target_bir_lowering=False, debug=False)

    xt_d = nc.declare_dram_parameter("xt", [D, F], b16, isOutput=False)
    yt_d = nc.declare_dram_parameter("yt", [D, T], b16, isOutput=False)
    mk_d = nc.declare_dram_parameter("maskT", [T, F], b16, isOutput=False)
    wq_d = nc.declare_dram_parameter("wq", [D, DG], b16, isOutput=False)
    wk_d = nc.declare_dram_parameter("wk", [D, DG], b16, isOutput=False)
    wv_d = nc.declare_dram_parameter("wv", [D, DG], b16, isOutput=False)
    bq_d = nc.declare_dram_parameter("bq", [128, MT], f32, isOutput=False)
    bk_d = nc.declare_dram_parameter("bk", [128, MT], f32, isOutput=False)
    bvr_d = nc.declare_dram_parameter("bvr", [1, DG], b16, isOutput=False)
    out_d = nc.declare_dram_parameter("out", [F, DG], b16, isOutput=True)

    EXPF = mybir.ActivationFunctionType.Exp

    with tile.TileContext(nc) as tc:
        with (
            tc.tile_pool(name="res", bufs=1) as res,
            tc.tile_pool(name="npool", bufs=2) as npool,
            tc.tile_pool(name="epool", bufs=KN("EPOOL", 5)) as epool,
            tc.tile_pool(name="empool", bufs=KN("EMPOOL", 12)) as empool,
            tc.tile_pool(name="spool", bufs=2, space="PSUM") as spool,
            tc.tile_pool(name="ppool", bufs=1, space="PSUM") as ppool,
            tc.tile_pool(name="cpool", bufs=1, space="PSUM") as cpool,
            tc.For_i(0, reps, 1) if reps > 1 else nullcontext(),
        ):
            # ---- resident SBUF ----
            xt_sb = res.tile([128, KT, F], b16, tag="xt")
            yt_sb = res.tile([128, KT, T], b16, tag="yt")
            mask_sb = res.tile([128, TT, F], b16, tag="mask")
            wq_sb = res.tile([128, KT, DG], b16, tag="wq")
            wk_sb = res.tile([128, KT, DG], b16, tag="wk")
            wv_sb = res.tile([128, KT, DG], b16, tag="wv")
            bq_sb = res.tile([128, MT], f32, tag="bq")
            bk_sb = res.tile([128, MT], f32, tag="bk")
            bvr_sb = res.tile([1, DG], b16, tag="bvr")
            ones = res.tile([1, 128], b16, tag="ones")
            qT_sb = res.tile([128, MT, F], b16, tag="qT")
            kT_sb = res.tile([128, MT, T], b16, tag="kT")
            v_sb = res.tile([128, TT, NH, VW], b16, tag="v")
            out_sb = res.tile([128, NCH, DG], b16, tag="out")

            nc.vector.memset(ones[:], 1.0)
            nc.vector.memset(v_sb[:, :, :, HEAD_DIM], 1.0)

            # ---- input DMAs (order = arrival order; one serial DMA stream) ----
            nc.sync.dma_start(wq_sb[:], wq_d.rearrange("(k p) n -> p k n", p=128))
            nc.sync.dma_start(wk_sb[:], wk_d.rearrange("(k p) n -> p k n", p=128))
            nc.sync.dma_start(bq_sb[:], bq_d[:])
            nc.sync.dma_start(bk_sb[:], bk_d[:])
            for k in range(KT):
                nc.sync.dma_start(xt_sb[:, k, :], xt_d[k * 128:(k + 1) * 128, :])
                nc.sync.dma_start(yt_sb[:, k, :], yt_d[k * 128:(k + 1) * 128, :])
            nc.sync.dma_start(wv_sb[:], wv_d.rearrange("(k p) n -> p k n", p=128))
            nc.sync.dma_start(bvr_sb[:], bvr_d[:])
            for t in range(TT):
                nc.sync.dma_start(mask_sb[:, t, :], mk_d[t * 128:(t + 1) * 128, :])

            # ---- projection emitters ----
            def qk_big(dst, w_sb, b_sb, act_sb, m, h0):
                # [128, HF] psum block in the scores pool (pre-loop only)
                ps = spool.tile([128, HF], f32, tag="s")
                for cs in range(0, HF, 512):
                    for k in range(KT):
                        nc.tensor.matmul(
                            ps[:, cs:cs + 512],
                            w_sb[:, k, m * 128:(m + 1) * 128],
                            act_sb[:, k, h0 + cs:h0 + cs + 512],
                            start=(k == 0), stop=(k == KT - 1))
                nc.vector.tensor_scalar_add(dst[:, m, h0:h0 + HF], ps[:],
                                            b_sb[:, m:m + 1])

            def qk_small(dst, w_sb, b_sb, act_sb, m, c0):
                # [128, 512] psum filler block (attention phase)
                ps = ppool.tile([128, 512], f32, tag="p")
                for k in range(KT):
                    nc.tensor.matmul(
                        ps[:], w_sb[:, k, m * 128:(m + 1) * 128],
                        act_sb[:, k, c0:c0 + 512],
                        start=(k == 0), stop=(k == KT - 1))
                nc.vector.tensor_scalar_add(dst[:, m, c0:c0 + 512], ps[:],
                                            b_sb[:, m:m + 1])

            def v_block(t):
                ps = ppool.tile([128, DG], f32, tag="p")
                for k in range(KT):
                    nc.tensor.matmul(
                        ps[:], yt_sb[:, k, t * 128:(t + 1) * 128], wv_sb[:, k, :],
                        start=(k == 0), stop=False)
                nc.tensor.matmul(ps[:], ones[0:1, :], bvr_sb[0:1, :],
                                 start=False, stop=True)
                nc.vector.tensor_copy(
                    v_sb[:, t, :, 0:HEAD_DIM],
                    ps.rearrange("p (h d) -> p h d", h=NH))

            # ---- pre-loop: q m0 + k m0 via the 1-bank proj pool ----
            # Deliberately NOT in the scores ring: in the steady reps-loop
            # state, the next rep's pre-loop must not WAR-block on this rep's
            # final exp reads of the scores psum slots; ppool frees mid-rep,
            # so the pre-loop hides under the previous rep's ACT-bound tail.
            for c0 in range(0, F, 512):
                qk_small(qT_sb, wq_sb, bq_sb, xt_sb, 0, c0)
            for c0 in range(0, T, 512):
                qk_small(kT_sb, wk_sb, bk_sb, yt_sb, 0, c0)

            # filler blocks consumed one per (h, t) iteration
            fillers = {
                0: [],
                1: ([lambda c0=c0: qk_small(qT_sb, wq_sb, bq_sb, xt_sb, 1, c0)
                     for c0 in range(0, F, 512)] +
                    [lambda c0=c0: qk_small(kT_sb, wk_sb, bk_sb, yt_sb, 1, c0)
                     for c0 in range(0, T, 512)]),
                2: [], 3: [],
            }
            # v blocks are position-constrained: v[t] emitted at iter t of head 0

            # ---- attention: flat 64-iteration pipeline ----
            # ctx matmuls trail the scores/exp/mask chain by CTX_DELAY iters
            # (so ctx of head h's last t-tiles spills into head h+1's early
            # iters), and each head's normalize is emitted while the NEXT
            # head's scores stream — PE and ACT never drain at boundaries.
            CTX_DELAY = KN("CTX_DELAY", 3)
            ctx_hold = [0]
            ctx_tiles = {}
            pend = []        # [(h, t, ems), ...] awaiting ctx emission
            norm_pend = []   # deferred normalize closures

            # ctx psum: 3 banks x 7 chunks of 65 f32 (matmul out must stay
            # inside one 2KB psum bank; accumulation groups are bank-granular:
            # start only on a bank's first matmul, stop only on its last).
            CPB = 7  # chunks per bank

            def chunk_ap(ctx_ps, c, w=VW):
                b, j = c // CPB, c % CPB
                return ctx_ps[:, b, j * VW:j * VW + w]

            def ctx_mms(hh, td, ems):
                ctx_ps = ctx_tiles[hh]
                for c in range(NCH):
                    em = ems[c // (NCH // 2)]
                    cc = c % (NCH // 2)
                    nc.tensor.matmul(
                        chunk_ap(ctx_ps, c),
                        em[:, cc * 128:(cc + 1) * 128],
                        v_sb[:, td, hh, :],
                        start=(td == 0 and c % CPB == 0),
                        stop=(td == TT - 1 and c in (6, 13, 15)))
                if td == TT - 1:
                    # head hh fully accumulated: reciprocal now, per-chunk
                    # normalizes spread over the following iterations
                    r_sb = npool.tile([128, NCH], f32, tag="r")
                    for b in range(3):
                        n = min(CPB, NCH - b * CPB)
                        den = ctx_ps[:, b, 0:n * VW].rearrange(
                            "p (c w) -> p c w", w=VW)[:, :, HEAD_DIM]
                        nc.vector.reciprocal(r_sb[:, b * CPB:b * CPB + n], den)

                    def norm_c(c, hh=hh, r_sb=r_sb, ctx_ps=ctx_ps):
                        nc.vector.tensor_scalar_mul(
                            out_sb[:, c, hh * HEAD_DIM:(hh + 1) * HEAD_DIM],
                            chunk_ap(ctx_ps, c, HEAD_DIM), r_sb[:, c:c + 1])
                    norm_pend.extend(
                        lambda c=c, f=norm_c: f(c) for c in range(NCH))
                    norm_pend.append(lambda hh=hh: nc.sync.dma_start(
                        out_d[:, hh * HEAD_DIM:(hh + 1) * HEAD_DIM].rearrange(
                            "(c p) n -> p c n", p=128),
                        out_sb[:, :, hh * HEAD_DIM:(hh + 1) * HEAD_DIM]))

            for h in range(NH):
                hp = (h % 2) * 64
                hm = h // 2
                ctx_tiles[h] = cpool.tile([128, 3, 512], f32, tag="ctx",
                                          name=f"ctx{h}")
                flist = list(fillers[h])
                for t in range(TT):
                    kh = kT_sb[hp:hp + 64, hm, t * 128:(t + 1) * 128]
                    ems = []
                    for half in range(2):
                        h0 = half * HF
                        s_ps = spool.tile([128, HF], f32, tag="s")
                        for cs in range(0, HF, 512):
                            nc.tensor.matmul(
                                s_ps[:, cs:cs + 512], kh,
                                qT_sb[hp:hp + 64, hm, h0 + cs:h0 + cs + 512],
                                start=True, stop=True)
                        e_sb = epool.tile([128, HF], b16, tag="e")
                        em_sb = empool.tile([128, HF], b16, tag="em")
                        nc.scalar.activation(e_sb[:], s_ps[:], EXPF, scale=0.125)
                        nc.vector.tensor_mul(em_sb[:], e_sb[:],
                                             mask_sb[:, t, h0:h0 + HF])
                        ems.append(em_sb)
                    # PE filler: v blocks during head 0, leftover proj spread
                    # every other iteration so no stretch goes PE-bound
                    if h == 0:
                        v_block(t)
                    if flist and (len(flist) * 2 > TT - t):
                        flist.pop(0)()
                    # ctx trails by CTX_DELAY; after a head's last ctx (which
                    # enqueues its normalizes) hold 2 iters so the normalize
                    # reads drain before the next head's start=True writes.
                    if ctx_hold[0] > 0:
                        ctx_hold[0] -= 1
                    else:
                        emitted = 0
                        while pend and len(pend) >= CTX_DELAY and emitted < 2:
                            entry = pend.pop(0)
                            ctx_mms(*entry)
                            emitted += 1
                            if entry[1] == TT - 1:
                                ctx_hold[0] = KN("HOLD", 2)
                                break
                    for _ in range(KN("NORM_RATE", 6)):
                        if norm_pend:
                            norm_pend.pop(0)()
                    pend.append((h, t, ems))
            while pend:
                ctx_mms(*pend.pop(0))
            while norm_pend:
                norm_pend.pop(0)()


    return nc


_CACHE = {}
TRACE = False


def _get_nc():
    if "nc" not in _CACHE:
        nc = build_nc()
        nc.compile()
        _CACHE["nc"] = nc
    return _CACHE["nc"]


def prep_in_maps(from_tensor, to_tensor, attention_mask, Wq, bq, Wk, bk, Wv, bv):
    from_tensor = np.asarray(from_tensor, np.float32)
    to_tensor = np.asarray(to_tensor, np.float32)
    attention_mask = np.asarray(attention_mask)
    in_maps = []
    for c in range(N_CORES):
        b, g = c // 4, c % 4
        sl = slice(g * 256, (g + 1) * 256)
        bqs = np.asarray(bq, np.float32)[sl]
        bks = np.asarray(bk, np.float32)[sl]
        bvs = np.asarray(bv, np.float32)[sl]
        in_maps.append({
            "xt": np.ascontiguousarray(from_tensor[b].T).astype(bf16),
            "yt": np.ascontiguousarray(to_tensor[b].T).astype(bf16),
            "maskT": np.ascontiguousarray(
                attention_mask[b].T.astype(np.float32)).astype(bf16),
            "wq": np.ascontiguousarray(np.asarray(Wq, np.float32)[:, sl]).astype(bf16),
            "wk": np.ascontiguousarray(np.asarray(Wk, np.float32)[:, sl]).astype(bf16),
            "wv": np.ascontiguousarray(np.asarray(Wv, np.float32)[:, sl]).astype(bf16),
            "bq": np.ascontiguousarray(bqs.reshape(2, 128).T),
            "bk": np.ascontiguousarray(bks.reshape(2, 128).T),
            "bvr": np.ascontiguousarray(bvs[None, :]).astype(bf16),
        })
    return in_maps


def gather_out(per_core_outs, B, F):
    out = np.zeros((B, F, HIDDEN), np.float32)
    for c in range(N_CORES):
        b, g = c // 4, c % 4
        out[b, :, g * 256:(g + 1) * 256] = np.asarray(per_core_outs[c],
                                                      dtype=np.float32)
    return out


def kernel(from_tensor, to_tensor, attention_mask, Wq, bq, Wk, bk, Wv, bv):
    from concourse.bass_utils import run_bass_kernel_spmd

    B, F, _ = np.asarray(from_tensor).shape
    nc = _get_nc()
    in_maps = prep_in_maps(from_tensor, to_tensor, attention_mask,
                           Wq, bq, Wk, bk, Wv, bv)
    res = run_bass_kernel_spmd(nc, in_maps, core_ids=list(range(N_CORES)),
                               trace=TRACE)
    _CACHE["last_result"] = res
    return gather_out([res.results[c]["out"] for c in range(N_CORES)], B, F)
